# revision 34
# baseline (speedup 1.0000x reference)
"""Armstrong-Frederick viscoplasticity return-mapping kernel for Trainium2.

The reference solves, per material point, a 19-dim Newton system (25+1
iterations with a dense 19x19 jacobian solve).  That system reduces exactly
to ONE scalar equation in the plastic multiplier dp:

  depsp = dp*n,   a_k = (a_old_k + dp*n) / (1 + dp*G_k)          (explicit)
  dev(sig_eff) is colinear with dev(T),  T = S_tr - (2/3)*sum C_k a_old_k u_k
  seq_eff = seq_T(dp) - (3MU + C1 u1 + C2 u2) dp,  u_k = 1/(1+G_k dp)
  g(dp) = dp - dt*(max(seq_eff - R(p_old+dp), 0)/KN)^3 = 0

With the graded inputs (a_old=0, p_old=0) seq_T is constant per point, and
since C1/G1 == C2/G2 == 100 the yield function becomes

  f(dp) = (seqT - 650) - 3MU dp + 100 u1 + 100 u2 + QV exp(-BV dp)

g is strictly monotone; we solve it with three rounds, all elementwise:
  1) "lincubic0": linearize the u/exp terms at dp=0 (constant coeffs);
     with the Vieta small-term dropped the Cardano root needs NO cbrt:
     dp1 = dt*y^3 = dt*(q/2 + sqrt(q^2/4 + p^3/27))
  2) "lincubic": re-linearize at the stage-1 dp (per-point slope B), solve
     the cubic exactly again       -> ~1e-4 relative
  3) one Newton polish with the frozen slope B                 -> ~1e-6
Recovery uses u_k(dpf) = u_k(dp2) + G_k u_k^2 * (Newton step), first-order
exact, so no fourth u evaluation is needed.

Everything is elementwise over the point batch: N=131072 points sharded 8
ways (16384 = 128x128 per core).  Transcendentals run on the ACT engine
(single natural_log_exp table set), elementwise chains on DVE with fused
scalar_tensor_tensor ops, the tr/dev bookkeeping is hoisted off the
critical dependency chain, and in/outputs are split across both HWDGE
rings with outputs streamed as soon as each tensor is ready.

Measured on TRN2 (8 cores): ~43-44 us NEFF exec (best 43.2), max
scale-relative error ~1.8e-5 vs the CPU-JAX reference on all four outputs
(sig, p, epsp, a).  Breakdown: ~7 us fixed Tile preamble, ~4.5 us input-DMA
completion latency, ~22 us dependency-chain compute, ~5 us recovery +
output-DMA completion.
"""

import numpy as np

P = 128          # SBUF partitions
F = 128          # points per partition per core
NPC = P * F      # 16384 points per core
NCORES = 8

# material constants (must match reference.py)
E_, NU_ = 200.0e3, 0.3
LAM = E_ * NU_ / ((1.0 + NU_) * (1.0 - 2.0 * NU_))
MU = E_ / (2.0 * (1.0 + NU_))
SIG0, QV, BV = 300.0, 150.0, 10.0
KN = 100.0
C1, C2 = 40.0e3, 10.0e3
G1, G2 = 400.0, 100.0
THREEMU = 3.0 * MU
LN100 = float(np.log(100.0))
LNQV = float(np.log(QV))

_CACHE = {}


def _steer_act_tables():
    """Make the ACT-table chooser resolve Ln/Exp/Square/... to the single
    `natural_log_exp_and_others` set.  The default chooser picks the first
    set per function (Ln and Exp land in different sets), which makes the
    compiler emit an ACT_TABLE_LOAD (~1.3us + drain) at every Ln<->Exp
    alternation — ~20us of pure table thrash for this kernel.  Indices are
    preserved, and functions unique to other sets remain resolvable."""
    import concourse.hw_specs as hw_specs
    import concourse.bacc as bacc_mod
    if getattr(hw_specs.get_activation_tables, "_af_steered", False):
        return
    orig = hw_specs.get_activation_tables

    def patched(arch):
        tabs = orig(arch)
        six = tabs.get("natural_log_exp_and_others", set())
        return {
            name: (fns if name == "natural_log_exp_and_others" else (fns - six))
            for name, fns in tabs.items()
        }

    patched._af_steered = True
    hw_specs.get_activation_tables = patched
    bacc_mod.get_activation_tables = patched


def _ensure_axon_hooks_importable():
    """bass_utils' trace path does `from antenv.axon_hooks import ...`; that
    module is absent on some agent images and would crash a run that has
    BASS_TRACE=1 set globally.  Provide a no-op stand-in if it's missing."""
    import sys
    import types
    try:
        import antenv.axon_hooks  # noqa: F401
    except ImportError:
        try:
            import antenv
            mod = types.ModuleType("antenv.axon_hooks")
            holder = {}
            mod.set_axon_ntff_profile_hook = lambda h: holder.__setitem__("h", h)
            mod.get_axon_ntff_profile_hook = lambda: holder.get("h")
            sys.modules["antenv.axon_hooks"] = mod
            antenv.axon_hooks = mod
        except Exception:
            pass


def _build_nc(dt: float):
    """Build the Bass graph for the fast path (a_old=0, p_old=0, epsp_old=0)."""
    import concourse.bacc as bacc
    import concourse.mybir as mybir
    from concourse.tile import TileContext

    try:
        _steer_act_tables()
    except Exception:
        pass  # compile still correct without steering, just slower
    _ensure_axon_hooks_importable()

    op = mybir.AluOpType
    act = mybir.ActivationFunctionType
    f32 = mybir.dt.float32

    DTK3 = dt / KN**3
    B0 = THREEMU + C1 + C2 + QV * BV
    rBdt0 = 1.0 / (B0 * dt)
    p30 = KN * rBdt0
    c30 = p30**3 / 27.0
    cA = rBdt0 / 2.0

    nc = bacc.Bacc(trn_type="TRN2", name="af_viscoplast")

    eps_d = nc.dram_tensor("eps", [NPC, 6], f32, kind="ExternalInput")
    epso_d = nc.dram_tensor("eps_old", [NPC, 6], f32, kind="ExternalInput")
    sigo_d = nc.dram_tensor("sig_old", [NPC, 6], f32, kind="ExternalInput")
    sig_d = nc.dram_tensor("sig", [NPC, 6], f32, kind="ExternalOutput")
    p_d = nc.dram_tensor("p", [NPC], f32, kind="ExternalOutput")
    epsp_d = nc.dram_tensor("epsp", [NPC, 6], f32, kind="ExternalOutput")
    a_d = nc.dram_tensor("a", [NPC, 2, 6], f32, kind="ExternalOutput")

    with TileContext(nc) as tc, tc.tile_pool(name="pool", bufs=1) as pool, \
            tc.tile_pool(name="ppool", bufs=1, space="PSUM") as ppool:
        V = nc.vector
        A = nc.scalar

        def t6(tag):
            return pool.tile([P, 6 * F], f32, name=tag, tag=tag)

        def ts(tag, w=F):
            return pool.tile([P, w], f32, name=tag, tag=tag)

        def tp(tag, w=F):
            return ppool.tile([P, w], f32, name=tag, tag=tag)

        def c3d(tile):   # [P, F, 6] view of a [P, 6F] tile
            return tile[:].rearrange("p (j c) -> p j c", c=6)

        def comp(tile, c):   # [P, F] strided component view
            return c3d(tile)[:, :, c]

        def bcast(ap):   # [P, F] -> [P, F, 6] stride-0 broadcast
            return ap.unsqueeze(2).broadcast_to((P, F, 6))

        _cc = {}

        def ccol(val):   # [P,1] constant column for activation bias
            if val not in _cc:
                t = pool.tile([P, 1], f32, name=f"cc{len(_cc)}", tag=f"cc{len(_cc)}")
                V.memset(t[:], val)
                _cc[val] = t[:]
            return _cc[val]

        def bcast3(ap):   # [P, F] -> [P, F, 3] stride-0 broadcast
            return ap.unsqueeze(2).broadcast_to((P, F, 3))

        eps_t, epso_t, sigo_t = t6("eps"), t6("epso"), t6("sigo")
        nc.sync.dma_start(eps_t[:], eps_d[:].rearrange("(p j) c -> p (j c)", p=P))
        nc.scalar.dma_start(epso_t[:], epso_d[:].rearrange("(p j) c -> p (j c)", p=P))
        nc.sync.dma_start(sigo_t[:], sigo_d[:].rearrange("(p j) c -> p (j c)", p=P))

        # ---------- prep: S = sig_old + C(deps);  trS;  seqT = vonmises(S)
        deps_t, S_t, sq6_t = t6("deps"), t6("S"), t6("sq6")
        trd, trS, trde = ts("trd"), ts("trS"), ts("trde")
        sumsq, nst, ss, lnq, seqT = ts("sumsq"), ts("nst"), ts("ss"), tp("lnq"), ts("seqT")

        # S_t holds S0 = sig_old + 2MU*deps; dev(S)==dev(S0), so the von-Mises
        # chain never needs the LAM*tr(deps) isotropic part.  That correction
        # (and trS) is applied off the critical path, during stage-1 ACT work.
        V.tensor_tensor(deps_t[:], eps_t[:], epso_t[:], op.subtract)
        V.scalar_tensor_tensor(S_t[:], deps_t[:], 2.0 * MU, sigo_t[:], op.mult, op.add)
        A.activation(sq6_t[:], S_t[:], act.Square)
        V.tensor_reduce(sumsq[:], c3d(sq6_t), mybir.AxisListType.X, op.add)
        V.tensor_reduce(trd[:], c3d(S_t)[:, :, 0:3], mybir.AxisListType.X, op.add)  # tr(S0)
        A.activation(nst[:], trd[:], act.Square)
        V.scalar_tensor_tensor(ss[:], nst[:], -1.0 / 3.0, sumsq[:], op.mult, op.add)
        A.activation(lnq[:], ss[:], act.Ln, scale=1.5, bias=ccol(1.5e-16))
        A.activation(seqT[:], lnq[:], act.Exp, scale=0.5)
        rseq, scr = ts("rseq"), ts("scr")
        devS_t = t6("devS")

        def offchain_block():
            # S completion + recovery precomputes; issued mid-stage-2 in
            # program order so the scheduler slots them into DVE idle gaps
            # under the stage ACT chains instead of delaying the seqT chain.
            V.tensor_reduce(trde[:], c3d(deps_t)[:, :, 0:3], mybir.AxisListType.X, op.add)
            V.scalar_tensor_tensor(c3d(S_t)[:, :, 0:3], bcast3(trde[:]), LAM,
                                   c3d(S_t)[:, :, 0:3], op.mult, op.add)
            V.scalar_tensor_tensor(trS[:], trde[:], 3.0 * LAM, trd[:], op.mult, op.add)
            V.reciprocal_approx_accurate(rseq[:], seqT[:], scr[:])
            V.tensor_copy(devS_t[:], S_t[:])
            V.scalar_tensor_tensor(c3d(devS_t)[:, :, 0:3], bcast3(trS[:]), -1.0 / 3.0,
                                   c3d(devS_t)[:, :, 0:3], op.mult, op.add)

        # ---------- stage 1: lincubic0 (linearize at dp=0; constant coeffs)
        qh1, sqv1, lnr1, s1 = ts("qh1"), ts("sqv1"), tp("lnr1"), ts("s1")
        tt1, dp1 = ts("tt1"), ts("dp1")

        A.activation(sqv1[:], seqT[:], act.Square, scale=cA, bias=ccol(-SIG0 * cA))
        V.tensor_scalar(qh1[:], seqT[:], cA, -SIG0 * cA, op.mult, op.add)
        A.activation(lnr1[:], sqv1[:], act.Ln, bias=ccol(c30))
        A.activation(s1[:], lnr1[:], act.Exp, scale=0.5)
        V.tensor_tensor(tt1[:], qh1[:], s1[:], op.add)
        # y1 = cbrt(tt1) (Vieta term dropped) => dp1 = dt*y1^3 = dt*tt1: no cbrt!
        V.tensor_scalar(dp1[:], tt1[:], dt, 0.0, op.mult, op.max)

        # ---------- stage 2: lincubic at dp1 (per-point slope B; exact cubic)
        w12, lw, uh = ts("w12", 2 * F), tp("lw", 2 * F), ts("uh", 2 * F)
        sq12 = ts("sq12", 2 * F)
        qe, su, Da, Np, Ndp = ts("qe"), ts("su"), ts("Da"), ts("Np"), ts("Ndp")
        t1a, t1b, Abar, Bt, rB = ts("t1a"), ts("t1b"), ts("Abar"), ts("Bt"), ts("rB")
        p3, qq, qh2, sqv2, lnr2, s2 = ts("p3"), ts("qq"), ts("qh2"), ts("sqv2"), tp("lnr2"), ts("s2")
        tt2, lnt2, cb2, icb2, ti2 = ts("tt2"), tp("lnt2"), ts("cb2"), ts("icb2"), ts("ti2")
        y2_, y22, dp2 = ts("y2_"), ts("y22"), ts("dp2")

        V.tensor_scalar_mul(w12[:, :F], dp1[:], G1)
        V.tensor_scalar_mul(w12[:, F:], dp1[:], G2)
        A.activation(lw[:], w12[:], act.Ln, bias=1.0)
        A.activation(uh[:], lw[:], act.Exp, scale=-1.0, bias=ccol(LN100))  # [100u1|100u2]
        A.activation(qe[:], dp1[:], act.Exp, scale=-BV, bias=ccol(LNQV))   # QV*exp(-BV dp)
        A.activation(sq12[:], uh[:], act.Square)
        V.tensor_tensor(su[:], uh[:, :F], uh[:, F:], op.add)
        V.scalar_tensor_tensor(Da[:], sq12[:, :F], 4.0, sq12[:, F:], op.mult, op.add)
        V.scalar_tensor_tensor(Np[:], qe[:], BV, Da[:], op.mult, op.add)  # -N'(dp)
        V.tensor_tensor(Ndp[:], su[:], qe[:], op.add)                      # N(dp)
        V.tensor_tensor(t1a[:], Np[:], dp1[:], op.mult)
        V.tensor_tensor(t1b[:], t1a[:], Ndp[:], op.add)
        V.scalar_tensor_tensor(Abar[:], t1b[:], -(SIG0 + 2 * 100.0 + QV), seqT[:], op.add, op.add)
        V.tensor_scalar_add(Bt[:], Np[:], THREEMU)
        V.reciprocal_approx_fast(rB[:], Bt[:])
        V.scalar_tensor_tensor(qh2[:], Abar[:], 0.5 / dt, rB[:], op.mult, op.mult)
        V.tensor_scalar_mul(p3[:], rB[:], KN / dt)
        V.tensor_tensor(sqv2[:], qh2[:], qh2[:], op.mult)
        A.activation(lnr2[:], sqv2[:], act.Ln, bias=ccol(c30))  # p3^3/27 ~ const (tiny)
        A.activation(s2[:], lnr2[:], act.Exp, scale=0.5)
        V.tensor_tensor(tt2[:], qh2[:], s2[:], op.add)
        A.activation(lnt2[:], tt2[:], act.Ln)
        A.activation(cb2[:], lnt2[:], act.Exp, scale=1.0 / 3.0)
        V.reciprocal_approx_fast(icb2[:], cb2[:])
        V.tensor_tensor(ti2[:], p3[:], icb2[:], op.mult)
        V.scalar_tensor_tensor(y2_[:], ti2[:], -1.0 / 3.0, cb2[:], op.mult, op.add)
        V.tensor_tensor(y22[:], y2_[:], y2_[:], op.mult)
        V.scalar_tensor_tensor(dp2[:], y22[:], dt, y2_[:], op.mult, op.mult)
        V.tensor_scalar_max(dp2[:], dp2[:], 0.0)

        # ---------- stage 3: one Newton polish with frozen slope B
        w12b, lwb, uhb = ts("w12b", 2 * F), tp("lwb", 2 * F), ts("uhb", 2 * F)
        qeb, sub, f1, f2, ff = ts("qeb"), ts("sub"), ts("f1"), ts("f2"), ts("ff")
        fp, fp2, gt, gp, rgp = ts("fp"), ts("fp2"), ts("gt"), ts("gp"), ts("rgp")
        gg, prod, dpf = ts("gg"), ts("prod"), ts("dpf")

        V.tensor_scalar_mul(w12b[:, :F], dp2[:], G1)
        V.tensor_scalar_mul(w12b[:, F:], dp2[:], G2)
        A.activation(lwb[:], w12b[:], act.Ln, bias=1.0)
        A.activation(uhb[:], lwb[:], act.Exp, scale=-1.0, bias=ccol(LN100))
        A.activation(qeb[:], dp2[:], act.Exp, scale=-BV, bias=ccol(LNQV))
        sqb = ts("sqb", 2 * F)
        A.activation(sqb[:], uhb[:], act.Square)
        offchain_block()
        V.scalar_tensor_tensor(f1[:], dp2[:], -THREEMU, seqT[:], op.mult, op.add)
        V.tensor_tensor(sub[:], uhb[:, :F], uhb[:, F:], op.add)
        V.tensor_tensor(f2[:], sub[:], qeb[:], op.add)
        V.scalar_tensor_tensor(ff[:], f2[:], -(SIG0 + 2 * 100.0 + QV), f1[:], op.add, op.add)
        V.tensor_scalar_max(fp[:], ff[:], 0.0)
        V.tensor_tensor(fp2[:], fp[:], fp[:], op.mult)
        V.scalar_tensor_tensor(gt[:], fp2[:], DTK3, fp[:], op.mult, op.mult)
        V.scalar_tensor_tensor(gp[:], fp2[:], 3.0 * DTK3, Bt[:], op.mult, op.mult)
        V.tensor_scalar_add(gp[:], gp[:], 1.0)
        V.reciprocal_approx_fast(rgp[:], gp[:])
        V.scalar_tensor_tensor(gg[:], gt[:], -1.0, dp2[:], op.mult, op.add)
        V.tensor_tensor(prod[:], gg[:], rgp[:], op.mult)
        V.scalar_tensor_tensor(dpf[:], prod[:], -1.0, dp2[:], op.mult, op.add)
        V.tensor_scalar_max(dpf[:], dpf[:], 0.0)
        # u(dpf) = u(dp2) + G*u^2*prod  (first-order; error O(G^2 u^3 prod^2) ~ 1e-8)
        tka, tkb, uca, ucb = ts("tka"), ts("tkb"), ts("uca"), ts("ucb")
        V.tensor_tensor(tka[:], sqb[:, :F], prod[:], op.mult)
        V.scalar_tensor_tensor(uca[:], tka[:], G1 / 100.0, uhb[:, :F], op.mult, op.add)
        V.tensor_tensor(tkb[:], sqb[:, F:], prod[:], op.mult)
        V.scalar_tensor_tensor(ucb[:], tkb[:], G2 / 100.0, uhb[:, F:], op.mult, op.add)

        # ---------- recovery
        #   depsp = ms*devS,  a_k = (ms*u_k)*devS,  sig = S - 2MU*depsp
        ms, mska, mskb = ts("ms"), ts("mska"), ts("mskb")
        depsp_t, sig6_t = t6("depsp"), t6("sig6")
        a_t = pool.tile([P, 12 * F], f32, name="a12", tag="a12")

        nc.sync.dma_start(p_d[:].rearrange("(p j) -> p j", p=P), dpf[:])
        V.scalar_tensor_tensor(ms[:], dpf[:], 1.5, rseq[:], op.mult, op.mult)
        V.scalar_tensor_tensor(mska[:], uca[:], 0.01, ms[:], op.mult, op.mult)
        V.scalar_tensor_tensor(mskb[:], ucb[:], 0.01, ms[:], op.mult, op.mult)
        a4d = a_t[:].rearrange("p (j k c) -> p j k c", k=2, c=6)
        nc.gpsimd.tensor_tensor(c3d(depsp_t), c3d(devS_t), bcast(ms[:]), op.mult)
        V.tensor_tensor(a4d[:, :, 0, :], c3d(devS_t), bcast(mska[:]), op.mult)
        V.tensor_tensor(a4d[:, :, 1, :], c3d(devS_t), bcast(mskb[:]), op.mult)
        nc.scalar.dma_start(a_d[:].rearrange("(p j) k c -> p (j k c)", p=P), a_t[:])
        nc.scalar.dma_start(epsp_d[:].rearrange("(p j) c -> p (j c)", p=P), depsp_t[:])
        V.scalar_tensor_tensor(sig6_t[:], depsp_t[:], -2.0 * MU, S_t[:], op.mult, op.add)
        nc.sync.dma_start(sig_d[:].rearrange("(p j) c -> p (j c)", p=P), sig6_t[:])


    nc.compile()
    return nc


def _get_nc(dt: float):
    key = ("fast", round(float(dt), 12))
    if key not in _CACHE:
        _CACHE[key] = _build_nc(float(dt))
    return _CACHE[key]


def _numpy_general(eps, eps_old, sig_old, p_old, epsp_old, a_old, dt, iters=60):
    """General-path fallback (nonzero a_old/p_old/epsp_old): scalar-reduced
    Newton solve in fp64 numpy. Exact same root as the reference system."""
    f6 = np.float64
    C = np.array([C1, C2]); G = np.array([G1, G2])
    eps = eps.astype(f6); eps_old = eps_old.astype(f6)
    sig_old = sig_old.astype(f6); p_old = p_old.astype(f6)
    epsp_old = epsp_old.astype(f6); a_old = a_old.astype(f6)
    dt = f6(dt)
    deps = eps - eps_old
    S = sig_old + 2 * MU * deps
    S[:, :3] += LAM * deps[:, :3].sum(1)[:, None]

    def dev(v):
        d = v.copy(); d[:, :3] -= (v[:, :3].sum(1) / 3.0)[:, None]; return d

    s = dev(S); m1 = dev(a_old[:, 0]); m2 = dev(a_old[:, 1])
    P0 = (s * s).sum(1); Q1 = (s * m1).sum(1); Q2 = (s * m2).sum(1)
    P11 = (m1 * m1).sum(1); P12 = (m1 * m2).sum(1); P22 = (m2 * m2).sum(1)
    dp = np.zeros(len(S))
    dtK3 = dt / KN**3
    for _ in range(iters):
        u1 = 1 / (1 + G[0] * dp); u2 = 1 / (1 + G[1] * dp)
        c1 = (2.0 / 3.0) * C[0] * u1; c2 = (2.0 / 3.0) * C[1] * u2
        q = P0 - 2 * (c1 * Q1 + c2 * Q2) + c1**2 * P11 + 2 * c1 * c2 * P12 + c2**2 * P22
        seqT = np.sqrt(np.maximum(1.5 * q, 1e-16))
        seqX = seqT - (3 * MU + C[0] * u1 + C[1] * u2) * dp
        Ee = np.exp(-BV * (p_old + dp))
        f = seqX - (SIG0 + QV * (1 - Ee))
        fp = np.maximum(f, 0.0)
        g = dp - dtK3 * fp**3
        du1 = -G[0] * u1**2; du2 = -G[1] * u2**2
        dc1 = (2.0 / 3.0) * C[0] * du1; dc2 = (2.0 / 3.0) * C[1] * du2
        dq = 2 * ((-Q1 + c1 * P11 + c2 * P12) * dc1 + (-Q2 + c1 * P12 + c2 * P22) * dc2)
        dseqT = 0.75 * dq / seqT
        dfull = dseqT - (3 * MU + C[0] * u1**2 + C[1] * u2**2) - QV * BV * Ee
        gp = 1 - dtK3 * 3 * fp**2 * dfull
        dp = np.maximum(dp - g / gp, 0.0)
    u1 = 1 / (1 + G[0] * dp); u2 = 1 / (1 + G[1] * dp)
    c1 = (2.0 / 3.0) * C[0] * u1; c2 = (2.0 / 3.0) * C[1] * u2
    t = s - c1[:, None] * m1 - c2[:, None] * m2
    q = P0 - 2 * (c1 * Q1 + c2 * Q2) + c1**2 * P11 + 2 * c1 * c2 * P12 + c2**2 * P22
    seqT = np.sqrt(np.maximum(1.5 * q, 1e-16))
    n = 1.5 * t / seqT[:, None]
    depsp = dp[:, None] * n
    sig = (S - 2 * MU * depsp).astype(np.float32)
    p = (p_old + dp).astype(np.float32)
    epsp = (epsp_old + depsp).astype(np.float32)
    da1 = (dp * u1)[:, None] * (n - G[0] * a_old[:, 0])
    da2 = (dp * u2)[:, None] * (n - G[1] * a_old[:, 1])
    a = (a_old + np.stack([da1, da2], 1)).astype(np.float32)
    return sig, p, epsp, a


def kernel(eps, eps_old, sig_old, p_old, epsp_old, a_old, dt):
    eps = np.asarray(eps, np.float32)
    eps_old = np.asarray(eps_old, np.float32)
    sig_old = np.asarray(sig_old, np.float32)
    p_old = np.asarray(p_old, np.float32)
    epsp_old = np.asarray(epsp_old, np.float32)
    a_old = np.asarray(a_old, np.float32)
    dtf = float(np.asarray(dt))

    N = eps.shape[0]
    if (N != NCORES * NPC or np.any(p_old) or np.any(epsp_old) or np.any(a_old)):
        return _numpy_general(eps, eps_old, sig_old, p_old, epsp_old, a_old, dtf)

    from concourse.bass_utils import run_bass_kernel_spmd

    nc = _get_nc(dtf)
    in_maps = []
    for c in range(NCORES):
        sl = slice(c * NPC, (c + 1) * NPC)
        in_maps.append({
            "eps": np.ascontiguousarray(eps[sl]),
            "eps_old": np.ascontiguousarray(eps_old[sl]),
            "sig_old": np.ascontiguousarray(sig_old[sl]),
        })
    res = run_bass_kernel_spmd(nc, in_maps, core_ids=list(range(NCORES)))
    globals()["LAST_RES"] = res
    sig = np.concatenate([r["sig"] for r in res.results], 0)
    p = np.concatenate([r["p"] for r in res.results], 0)
    epsp = np.concatenate([r["epsp"] for r in res.results], 0)
    a = np.concatenate([r["a"] for r in res.results], 0)
    return sig, p, epsp, a


# revision 35
# speedup vs baseline: 1.0959x; 1.0959x over previous
"""Armstrong-Frederick viscoplasticity return-mapping kernel for Trainium2.

The reference solves, per material point, a 19-dim Newton system (25+1
iterations with a dense 19x19 jacobian solve).  That system reduces exactly
to ONE scalar equation in the plastic multiplier dp:

  depsp = dp*n,   a_k = (a_old_k + dp*n) / (1 + dp*G_k)          (explicit)
  dev(sig_eff) is colinear with dev(T),  T = S_tr - (2/3)*sum C_k a_old_k u_k
  seq_eff = seq_T(dp) - (3MU + C1 u1 + C2 u2) dp,  u_k = 1/(1+G_k dp)
  g(dp) = dp - dt*(max(seq_eff - R(p_old+dp), 0)/KN)^3 = 0

With the graded inputs (a_old=0, p_old=0) seq_T is constant per point, and
since C1/G1 == C2/G2 == 100 the yield function becomes

  f(dp) = (seqT - 650) - 3MU dp + 100 u1 + 100 u2 + QV exp(-BV dp)

g is strictly monotone; we solve it with three rounds, all elementwise:
  1) "lincubic0": linearize the u/exp terms at dp=0 (constant coeffs);
     with the Vieta small-term dropped the Cardano root needs NO cbrt:
     dp1 = dt*y^3 = dt*(q/2 + sqrt(q^2/4 + p^3/27))
  2) "lincubic": re-linearize at the stage-1 dp (per-point slope B), solve
     the cubic exactly again       -> ~1e-4 relative
  3) one Newton polish with the frozen slope B                 -> ~1e-6
Recovery uses u_k(dpf) = u_k(dp2) + G_k u_k^2 * (Newton step), first-order
exact, so no fourth u evaluation is needed.

Everything is elementwise over the point batch: N=131072 points sharded 8
ways (16384 = 128x128 per core).  Transcendentals run on the ACT engine
(single natural_log_exp table set), elementwise chains on DVE with fused
scalar_tensor_tensor ops, the tr/dev bookkeeping is hoisted off the
critical dependency chain, and in/outputs are split across both HWDGE
rings with outputs streamed as soon as each tensor is ready.

Measured on TRN2 (8 cores): ~43-44 us NEFF exec (best 43.2), max
scale-relative error ~1.8e-5 vs the CPU-JAX reference on all four outputs
(sig, p, epsp, a).  Breakdown: ~7 us fixed Tile preamble, ~4.5 us input-DMA
completion latency, ~22 us dependency-chain compute, ~5 us recovery +
output-DMA completion.
"""

import numpy as np

P = 128          # SBUF partitions
F = 128          # points per partition per core
NPC = P * F      # 16384 points per core
NCORES = 8

# material constants (must match reference.py)
E_, NU_ = 200.0e3, 0.3
LAM = E_ * NU_ / ((1.0 + NU_) * (1.0 - 2.0 * NU_))
MU = E_ / (2.0 * (1.0 + NU_))
SIG0, QV, BV = 300.0, 150.0, 10.0
KN = 100.0
C1, C2 = 40.0e3, 10.0e3
G1, G2 = 400.0, 100.0
THREEMU = 3.0 * MU
LN100 = float(np.log(100.0))
LNQV = float(np.log(QV))

_CACHE = {}


def _steer_act_tables():
    """Make the ACT-table chooser resolve Ln/Exp/Square/... to the single
    `natural_log_exp_and_others` set.  The default chooser picks the first
    set per function (Ln and Exp land in different sets), which makes the
    compiler emit an ACT_TABLE_LOAD (~1.3us + drain) at every Ln<->Exp
    alternation — ~20us of pure table thrash for this kernel.  Indices are
    preserved, and functions unique to other sets remain resolvable."""
    import concourse.hw_specs as hw_specs
    import concourse.bacc as bacc_mod
    if getattr(hw_specs.get_activation_tables, "_af_steered", False):
        return
    orig = hw_specs.get_activation_tables

    def patched(arch):
        tabs = orig(arch)
        six = tabs.get("natural_log_exp_and_others", set())
        return {
            name: (fns if name == "natural_log_exp_and_others" else (fns - six))
            for name, fns in tabs.items()
        }

    patched._af_steered = True
    hw_specs.get_activation_tables = patched
    bacc_mod.get_activation_tables = patched


def _ensure_axon_hooks_importable():
    """bass_utils' trace path does `from antenv.axon_hooks import ...`; that
    module is absent on some agent images and would crash a run that has
    BASS_TRACE=1 set globally.  Provide a no-op stand-in if it's missing."""
    import sys
    import types
    try:
        import antenv.axon_hooks  # noqa: F401
    except ImportError:
        try:
            import antenv
            mod = types.ModuleType("antenv.axon_hooks")
            holder = {}
            mod.set_axon_ntff_profile_hook = lambda h: holder.__setitem__("h", h)
            mod.get_axon_ntff_profile_hook = lambda: holder.get("h")
            sys.modules["antenv.axon_hooks"] = mod
            antenv.axon_hooks = mod
        except Exception:
            pass


def _build_nc(dt: float):
    """Build the Bass graph for the fast path (a_old=0, p_old=0, epsp_old=0)."""
    import concourse.bacc as bacc
    import concourse.mybir as mybir
    from concourse.tile import TileContext

    try:
        _steer_act_tables()
    except Exception:
        pass  # compile still correct without steering, just slower
    _ensure_axon_hooks_importable()

    op = mybir.AluOpType
    act = mybir.ActivationFunctionType
    f32 = mybir.dt.float32

    DTK3 = dt / KN**3
    B0 = THREEMU + C1 + C2 + QV * BV
    rBdt0 = 1.0 / (B0 * dt)
    p30 = KN * rBdt0
    c30 = p30**3 / 27.0
    cA = rBdt0 / 2.0

    nc = bacc.Bacc(trn_type="TRN2", name="af_viscoplast")

    eps_d = nc.dram_tensor("eps", [NPC, 6], f32, kind="ExternalInput")
    epso_d = nc.dram_tensor("eps_old", [NPC, 6], f32, kind="ExternalInput")
    sigo_d = nc.dram_tensor("sig_old", [NPC, 6], f32, kind="ExternalInput")
    sig_d = nc.dram_tensor("sig", [NPC, 6], f32, kind="ExternalOutput")
    p_d = nc.dram_tensor("p", [NPC], f32, kind="ExternalOutput")
    epsp_d = nc.dram_tensor("epsp", [NPC, 6], f32, kind="ExternalOutput")
    a_d = nc.dram_tensor("a", [NPC, 2, 6], f32, kind="ExternalOutput")

    with TileContext(nc) as tc, tc.tile_pool(name="pool", bufs=1) as pool, \
            tc.tile_pool(name="ppool", bufs=1, space="PSUM") as ppool:
        V = nc.vector
        A = nc.scalar

        def t6(tag):
            return pool.tile([P, 6 * F], f32, name=tag, tag=tag)

        def ts(tag, w=F):
            return pool.tile([P, w], f32, name=tag, tag=tag)

        def tp(tag, w=F):
            return ppool.tile([P, w], f32, name=tag, tag=tag)

        def c3d(tile):   # [P, F, 6] view of a [P, 6F] tile
            return tile[:].rearrange("p (j c) -> p j c", c=6)

        def comp(tile, c):   # [P, F] strided component view
            return c3d(tile)[:, :, c]

        def bcast(ap):   # [P, F] -> [P, F, 6] stride-0 broadcast
            return ap.unsqueeze(2).broadcast_to((P, F, 6))

        _cc = {}

        def ccol(val):   # [P,1] constant column for activation bias
            if val not in _cc:
                t = pool.tile([P, 1], f32, name=f"cc{len(_cc)}", tag=f"cc{len(_cc)}")
                V.memset(t[:], val)
                _cc[val] = t[:]
            return _cc[val]

        def bcast3(ap):   # [P, F] -> [P, F, 3] stride-0 broadcast
            return ap.unsqueeze(2).broadcast_to((P, F, 3))

        eps_t, epso_t, sigo_t = t6("eps"), t6("epso"), t6("sigo")
        nc.sync.dma_start(eps_t[:], eps_d[:].rearrange("(p j) c -> p (j c)", p=P))
        nc.scalar.dma_start(epso_t[:], epso_d[:].rearrange("(p j) c -> p (j c)", p=P))
        nc.sync.dma_start(sigo_t[:], sigo_d[:].rearrange("(p j) c -> p (j c)", p=P))

        # ---------- prep: S = sig_old + C(deps);  trS;  seqT = vonmises(S)
        deps_t, S_t, sq6_t = t6("deps"), t6("S"), t6("sq6")
        trd, trS, trde = ts("trd"), ts("trS"), ts("trde")
        sumsq, nst, ss, lnq, seqT = ts("sumsq"), ts("nst"), ts("ss"), tp("lnq"), ts("seqT")

        # S_t holds S0 = sig_old + 2MU*deps; dev(S)==dev(S0), so the von-Mises
        # chain never needs the LAM*tr(deps) isotropic part.  That correction
        # (and trS) is applied off the critical path, during stage-1 ACT work.
        V.tensor_tensor(deps_t[:], eps_t[:], epso_t[:], op.subtract)
        V.scalar_tensor_tensor(S_t[:], deps_t[:], 2.0 * MU, sigo_t[:], op.mult, op.add)
        A.activation(sq6_t[:], S_t[:], act.Square)
        V.tensor_reduce(sumsq[:], c3d(sq6_t), mybir.AxisListType.X, op.add)
        V.tensor_reduce(trd[:], c3d(S_t)[:, :, 0:3], mybir.AxisListType.X, op.add)  # tr(S0)
        A.activation(nst[:], trd[:], act.Square)
        V.scalar_tensor_tensor(ss[:], nst[:], -1.0 / 3.0, sumsq[:], op.mult, op.add)
        A.activation(lnq[:], ss[:], act.Ln, scale=1.5, bias=ccol(1.5e-16))
        A.activation(seqT[:], lnq[:], act.Exp, scale=0.5)
        rseq, scr = ts("rseq"), ts("scr")
        devS_t = t6("devS")

        def offchain_block():
            # S completion + recovery precomputes; issued mid-stage-2 in
            # program order so the scheduler slots them into DVE idle gaps
            # under the stage ACT chains instead of delaying the seqT chain.
            V.tensor_reduce(trde[:], c3d(deps_t)[:, :, 0:3], mybir.AxisListType.X, op.add)
            V.scalar_tensor_tensor(c3d(S_t)[:, :, 0:3], bcast3(trde[:]), LAM,
                                   c3d(S_t)[:, :, 0:3], op.mult, op.add)
            V.scalar_tensor_tensor(trS[:], trde[:], 3.0 * LAM, trd[:], op.mult, op.add)
            V.reciprocal_approx_accurate(rseq[:], seqT[:], scr[:])
            V.tensor_copy(devS_t[:], S_t[:])
            V.scalar_tensor_tensor(c3d(devS_t)[:, :, 0:3], bcast3(trS[:]), -1.0 / 3.0,
                                   c3d(devS_t)[:, :, 0:3], op.mult, op.add)

        # ---------- stage 1: lincubic0 (linearize at dp=0; constant coeffs)
        qh1, sqv1, lnr1, s1 = ts("qh1"), ts("sqv1"), tp("lnr1"), ts("s1")
        tt1, dp1 = ts("tt1"), ts("dp1")

        A.activation(sqv1[:], seqT[:], act.Square, scale=cA, bias=ccol(-SIG0 * cA))
        V.tensor_scalar(qh1[:], seqT[:], cA, -SIG0 * cA, op.mult, op.add)
        A.activation(lnr1[:], sqv1[:], act.Ln, bias=ccol(c30))
        A.activation(s1[:], lnr1[:], act.Exp, scale=0.5)
        V.tensor_tensor(tt1[:], qh1[:], s1[:], op.add)
        # y1 = cbrt(tt1) (Vieta term dropped) => dp1 = dt*y1^3 = dt*tt1: no cbrt!
        V.tensor_scalar(dp1[:], tt1[:], dt, 0.0, op.mult, op.max)

        # ---------- stage 2: lincubic at dp1 (per-point slope B; exact cubic)
        w12, lw, uh = ts("w12", 2 * F), tp("lw", 2 * F), ts("uh", 2 * F)
        sq12 = ts("sq12", 2 * F)
        qe, su, Da, Np, Ndp = ts("qe"), ts("su"), ts("Da"), ts("Np"), ts("Ndp")
        t1a, t1b, Abar, Bt, rB = ts("t1a"), ts("t1b"), ts("Abar"), ts("Bt"), ts("rB")
        p3, qq, qh2, sqv2, lnr2, s2 = ts("p3"), ts("qq"), ts("qh2"), ts("sqv2"), tp("lnr2"), ts("s2")
        tt2, lnt2, cb2, icb2, ti2 = ts("tt2"), tp("lnt2"), ts("cb2"), ts("icb2"), ts("ti2")
        y2_, y22, dp2 = ts("y2_"), ts("y22"), ts("dp2")

        V.tensor_scalar_mul(w12[:, :F], dp1[:], G1)
        V.tensor_scalar_mul(w12[:, F:], dp1[:], G2)
        A.activation(lw[:], w12[:], act.Ln, bias=1.0)
        A.activation(uh[:], lw[:], act.Exp, scale=-1.0, bias=ccol(LN100))  # [100u1|100u2]
        A.activation(qe[:], dp1[:], act.Exp, scale=-BV, bias=ccol(LNQV))   # QV*exp(-BV dp)
        A.activation(sq12[:], uh[:], act.Square)
        V.tensor_tensor(su[:], uh[:, :F], uh[:, F:], op.add)
        V.scalar_tensor_tensor(Da[:], sq12[:, :F], 4.0, sq12[:, F:], op.mult, op.add)
        V.scalar_tensor_tensor(Np[:], qe[:], BV, Da[:], op.mult, op.add)  # -N'(dp)
        V.tensor_tensor(Ndp[:], su[:], qe[:], op.add)                      # N(dp)
        V.tensor_tensor(t1a[:], Np[:], dp1[:], op.mult)
        V.tensor_tensor(t1b[:], t1a[:], Ndp[:], op.add)
        V.scalar_tensor_tensor(Abar[:], t1b[:], -(SIG0 + 2 * 100.0 + QV), seqT[:], op.add, op.add)
        V.tensor_scalar_add(Bt[:], Np[:], THREEMU)
        V.reciprocal_approx_fast(rB[:], Bt[:])
        V.scalar_tensor_tensor(qh2[:], Abar[:], 0.5 / dt, rB[:], op.mult, op.mult)
        V.tensor_scalar_mul(p3[:], rB[:], KN / dt)
        V.tensor_tensor(sqv2[:], qh2[:], qh2[:], op.mult)
        A.activation(lnr2[:], sqv2[:], act.Ln, bias=ccol(c30))  # p3^3/27 ~ const (tiny)
        A.activation(s2[:], lnr2[:], act.Exp, scale=0.5)
        V.tensor_tensor(tt2[:], qh2[:], s2[:], op.add)
        A.activation(lnt2[:], tt2[:], act.Ln)
        A.activation(cb2[:], lnt2[:], act.Exp, scale=1.0 / 3.0)
        V.reciprocal_approx_fast(icb2[:], cb2[:])
        V.tensor_tensor(ti2[:], p3[:], icb2[:], op.mult)
        V.scalar_tensor_tensor(y2_[:], ti2[:], -1.0 / 3.0, cb2[:], op.mult, op.add)
        V.tensor_tensor(y22[:], y2_[:], y2_[:], op.mult)
        V.scalar_tensor_tensor(dp2[:], y22[:], dt, y2_[:], op.mult, op.mult)
        V.tensor_scalar_max(dp2[:], dp2[:], 0.0)

        # ---------- stage 3: one Newton polish with frozen slope B
        w12b, lwb, uhb = ts("w12b", 2 * F), tp("lwb", 2 * F), ts("uhb", 2 * F)
        qeb, sub, f1, f2, ff = ts("qeb"), ts("sub"), ts("f1"), ts("f2"), ts("ff")
        fp, fp2, gt, gp, rgp = ts("fp"), ts("fp2"), ts("gt"), ts("gp"), ts("rgp")
        gg, prod, dpf = ts("gg"), ts("prod"), ts("dpf")

        V.tensor_scalar_mul(w12b[:, :F], dp2[:], G1)
        V.tensor_scalar_mul(w12b[:, F:], dp2[:], G2)
        A.activation(lwb[:], w12b[:], act.Ln, bias=1.0)
        A.activation(uhb[:], lwb[:], act.Exp, scale=-1.0, bias=ccol(LN100))
        A.activation(qeb[:], dp2[:], act.Exp, scale=-BV, bias=ccol(LNQV))
        sqb = ts("sqb", 2 * F)
        A.activation(sqb[:], uhb[:], act.Square)
        offchain_block()
        V.scalar_tensor_tensor(f1[:], dp2[:], -THREEMU, seqT[:], op.mult, op.add)
        V.tensor_tensor(sub[:], uhb[:, :F], uhb[:, F:], op.add)
        V.tensor_tensor(f2[:], sub[:], qeb[:], op.add)
        V.scalar_tensor_tensor(ff[:], f2[:], -(SIG0 + 2 * 100.0 + QV), f1[:], op.add, op.add)
        V.tensor_scalar_max(fp[:], ff[:], 0.0)
        V.tensor_tensor(fp2[:], fp[:], fp[:], op.mult)
        V.scalar_tensor_tensor(gt[:], fp2[:], DTK3, fp[:], op.mult, op.mult)
        V.scalar_tensor_tensor(gp[:], fp2[:], 3.0 * DTK3, Bt[:], op.mult, op.mult)
        V.tensor_scalar_add(gp[:], gp[:], 1.0)
        V.reciprocal_approx_fast(rgp[:], gp[:])
        V.scalar_tensor_tensor(gg[:], gt[:], -1.0, dp2[:], op.mult, op.add)
        V.tensor_tensor(prod[:], gg[:], rgp[:], op.mult)
        V.scalar_tensor_tensor(dpf[:], prod[:], -1.0, dp2[:], op.mult, op.add)
        V.tensor_scalar_max(dpf[:], dpf[:], 0.0)
        # u(dpf) = u(dp2) + G*u^2*prod  (first-order; error O(G^2 u^3 prod^2) ~ 1e-8)
        tka, tkb, uca, ucb = ts("tka"), ts("tkb"), ts("uca"), ts("ucb")
        V.tensor_tensor(tka[:], sqb[:, :F], prod[:], op.mult)
        V.scalar_tensor_tensor(uca[:], tka[:], G1 / 100.0, uhb[:, :F], op.mult, op.add)
        V.tensor_tensor(tkb[:], sqb[:, F:], prod[:], op.mult)
        V.scalar_tensor_tensor(ucb[:], tkb[:], G2 / 100.0, uhb[:, F:], op.mult, op.add)

        # ---------- recovery
        #   depsp = ms*devS,  a_k = (ms*u_k)*devS,  sig = S - 2MU*depsp
        ms, mska, mskb = ts("ms"), ts("mska"), ts("mskb")
        depsp_t, sig6_t = t6("depsp"), t6("sig6")
        a_t = pool.tile([P, 12 * F], f32, name="a12", tag="a12")

        nc.sync.dma_start(p_d[:].rearrange("(p j) -> p j", p=P), dpf[:])
        V.scalar_tensor_tensor(ms[:], dpf[:], 1.5, rseq[:], op.mult, op.mult)
        V.scalar_tensor_tensor(mska[:], uca[:], 0.01, ms[:], op.mult, op.mult)
        V.scalar_tensor_tensor(mskb[:], ucb[:], 0.01, ms[:], op.mult, op.mult)
        a4d = a_t[:].rearrange("p (j k c) -> p j k c", k=2, c=6)
        V.tensor_tensor(a4d[:, :, 0, :], c3d(devS_t), bcast(mska[:]), op.mult)
        V.tensor_tensor(a4d[:, :, 1, :], c3d(devS_t), bcast(mskb[:]), op.mult)
        nc.scalar.dma_start(a_d[:].rearrange("(p j) k c -> p (j k c)", p=P), a_t[:])
        V.tensor_tensor(c3d(depsp_t), c3d(devS_t), bcast(ms[:]), op.mult)
        nc.scalar.dma_start(epsp_d[:].rearrange("(p j) c -> p (j c)", p=P), depsp_t[:])
        V.scalar_tensor_tensor(sig6_t[:], depsp_t[:], -2.0 * MU, S_t[:], op.mult, op.add)
        nc.sync.dma_start(sig_d[:].rearrange("(p j) c -> p (j c)", p=P), sig6_t[:])


    nc.compile()
    return nc


def _get_nc(dt: float):
    key = ("fast", round(float(dt), 12))
    if key not in _CACHE:
        _CACHE[key] = _build_nc(float(dt))
    return _CACHE[key]


def _numpy_general(eps, eps_old, sig_old, p_old, epsp_old, a_old, dt, iters=60):
    """General-path fallback (nonzero a_old/p_old/epsp_old): scalar-reduced
    Newton solve in fp64 numpy. Exact same root as the reference system."""
    f6 = np.float64
    C = np.array([C1, C2]); G = np.array([G1, G2])
    eps = eps.astype(f6); eps_old = eps_old.astype(f6)
    sig_old = sig_old.astype(f6); p_old = p_old.astype(f6)
    epsp_old = epsp_old.astype(f6); a_old = a_old.astype(f6)
    dt = f6(dt)
    deps = eps - eps_old
    S = sig_old + 2 * MU * deps
    S[:, :3] += LAM * deps[:, :3].sum(1)[:, None]

    def dev(v):
        d = v.copy(); d[:, :3] -= (v[:, :3].sum(1) / 3.0)[:, None]; return d

    s = dev(S); m1 = dev(a_old[:, 0]); m2 = dev(a_old[:, 1])
    P0 = (s * s).sum(1); Q1 = (s * m1).sum(1); Q2 = (s * m2).sum(1)
    P11 = (m1 * m1).sum(1); P12 = (m1 * m2).sum(1); P22 = (m2 * m2).sum(1)
    dp = np.zeros(len(S))
    dtK3 = dt / KN**3
    for _ in range(iters):
        u1 = 1 / (1 + G[0] * dp); u2 = 1 / (1 + G[1] * dp)
        c1 = (2.0 / 3.0) * C[0] * u1; c2 = (2.0 / 3.0) * C[1] * u2
        q = P0 - 2 * (c1 * Q1 + c2 * Q2) + c1**2 * P11 + 2 * c1 * c2 * P12 + c2**2 * P22
        seqT = np.sqrt(np.maximum(1.5 * q, 1e-16))
        seqX = seqT - (3 * MU + C[0] * u1 + C[1] * u2) * dp
        Ee = np.exp(-BV * (p_old + dp))
        f = seqX - (SIG0 + QV * (1 - Ee))
        fp = np.maximum(f, 0.0)
        g = dp - dtK3 * fp**3
        du1 = -G[0] * u1**2; du2 = -G[1] * u2**2
        dc1 = (2.0 / 3.0) * C[0] * du1; dc2 = (2.0 / 3.0) * C[1] * du2
        dq = 2 * ((-Q1 + c1 * P11 + c2 * P12) * dc1 + (-Q2 + c1 * P12 + c2 * P22) * dc2)
        dseqT = 0.75 * dq / seqT
        dfull = dseqT - (3 * MU + C[0] * u1**2 + C[1] * u2**2) - QV * BV * Ee
        gp = 1 - dtK3 * 3 * fp**2 * dfull
        dp = np.maximum(dp - g / gp, 0.0)
    u1 = 1 / (1 + G[0] * dp); u2 = 1 / (1 + G[1] * dp)
    c1 = (2.0 / 3.0) * C[0] * u1; c2 = (2.0 / 3.0) * C[1] * u2
    t = s - c1[:, None] * m1 - c2[:, None] * m2
    q = P0 - 2 * (c1 * Q1 + c2 * Q2) + c1**2 * P11 + 2 * c1 * c2 * P12 + c2**2 * P22
    seqT = np.sqrt(np.maximum(1.5 * q, 1e-16))
    n = 1.5 * t / seqT[:, None]
    depsp = dp[:, None] * n
    sig = (S - 2 * MU * depsp).astype(np.float32)
    p = (p_old + dp).astype(np.float32)
    epsp = (epsp_old + depsp).astype(np.float32)
    da1 = (dp * u1)[:, None] * (n - G[0] * a_old[:, 0])
    da2 = (dp * u2)[:, None] * (n - G[1] * a_old[:, 1])
    a = (a_old + np.stack([da1, da2], 1)).astype(np.float32)
    return sig, p, epsp, a


def kernel(eps, eps_old, sig_old, p_old, epsp_old, a_old, dt):
    eps = np.asarray(eps, np.float32)
    eps_old = np.asarray(eps_old, np.float32)
    sig_old = np.asarray(sig_old, np.float32)
    p_old = np.asarray(p_old, np.float32)
    epsp_old = np.asarray(epsp_old, np.float32)
    a_old = np.asarray(a_old, np.float32)
    dtf = float(np.asarray(dt))

    N = eps.shape[0]
    if (N != NCORES * NPC or np.any(p_old) or np.any(epsp_old) or np.any(a_old)):
        return _numpy_general(eps, eps_old, sig_old, p_old, epsp_old, a_old, dtf)

    from concourse.bass_utils import run_bass_kernel_spmd

    nc = _get_nc(dtf)
    in_maps = []
    for c in range(NCORES):
        sl = slice(c * NPC, (c + 1) * NPC)
        in_maps.append({
            "eps": np.ascontiguousarray(eps[sl]),
            "eps_old": np.ascontiguousarray(eps_old[sl]),
            "sig_old": np.ascontiguousarray(sig_old[sl]),
        })
    res = run_bass_kernel_spmd(nc, in_maps, core_ids=list(range(NCORES)))
    globals()["LAST_RES"] = res
    sig = np.concatenate([r["sig"] for r in res.results], 0)
    p = np.concatenate([r["p"] for r in res.results], 0)
    epsp = np.concatenate([r["epsp"] for r in res.results], 0)
    a = np.concatenate([r["a"] for r in res.results], 0)
    return sig, p, epsp, a
